# revision 9
# baseline (speedup 1.0000x reference)
"""Trainium2 Bass kernel for nn_Block_78864189489800 (dense transformer block
with edge-conditioned attention).

Sharding: rows of the sequence (i dimension) are striped across the 8
NeuronCores (core c owns rows i with i % 8 == c, 48 rows each).  Every core
redundantly computes LN1 / K / V (cheap), and computes its own rows through
attention, projection, LN2 and the MLP.  No collectives are needed; the host
reassembles the 8 row-slices.

Key algebraic restructuring: the (T,T,C) edge tensor  ee = edge_emb[bias_matrix]
has only E=16 distinct rows, so
    e_k = (ee @ W_ek.T + b)  ==  tab_k[bias_matrix]   with tab_k = edge_emb @ W_ek.T + b
and the score einsum becomes, per edge type e:
    S_e[h,i,j] = sum_d (q[h,i,d] * tab_k[e,h,d]) * k[h,j,d]
with the (i,j) positions selected by a host-precomputed one-hot mask (causal
mask folded in).  Likewise the value-side einsum becomes a per-e matmul with
the per-e diagonal scale tab_v[e,h,:] (and the softmax bias exp(ab[e,h]))
applied after the j-contraction.  The softmax denominator comes for free from
an appended ones-column on V.
"""

import math

import numpy as np
import ml_dtypes

import concourse.bass as bass
import concourse.mybir as mybir
import concourse.tile as tile
from concourse import bacc
from concourse.bass_utils import run_bass_kernel_spmd
from concourse.masks import make_identity

# Problem shape (hardcoded per contract)
B, T, C, H, E = 1, 384, 512, 8, 16
D = C // H            # 64
NC = 8                # cores
R = T // NC           # 48 rows per core
P = 128
CCH = C // P          # 4 chunks of the C dim
NJB = T // P          # 3 j-blocks
F = 4 * C             # 2048
NRC = F // P          # 16 mlp row chunks
FP32 = mybir.dt.float32
BF16 = mybir.dt.bfloat16
AF = mybir.ActivationFunctionType
OP = mybir.AluOpType
BF16_NP = ml_dtypes.bfloat16

_prog_cache = {}


def _ln_transposed(nc, pools, xT_sb, n, lnw_sb, lnb_sb, out_sb, ones_f32,
                   eps_sb, tag):
    """LayerNorm with C on partitions: xT_sb (128,4,n) f32 -> out_sb (128,4,n) bf16.
    Stats via PE ones-matmuls (sum over partitions)."""
    ps_pool, sb_pool = pools
    ps_sx = ps_pool.tile([1, n], FP32, tag="m")
    ps_sx2 = ps_pool.tile([1, n], FP32, tag="m")
    xsq = sb_pool.tile([P, CCH, n], FP32, tag=f"lnxsq{tag}")
    for cc in range(CCH):
        nc.scalar.square(xsq[:, cc, :], xT_sb[:, cc, :])
    for cc in range(CCH):
        nc.tensor.matmul(ps_sx, ones_f32, xT_sb[:, cc, :],
                         start=(cc == 0), stop=(cc == CCH - 1))
    for cc in range(CCH):
        nc.tensor.matmul(ps_sx2, ones_f32, xsq[:, cc, :],
                         start=(cc == 0), stop=(cc == CCH - 1))
    mu = sb_pool.tile([1, n], FP32, tag=f"lnmu{tag}")
    nc.vector.tensor_scalar_mul(mu, ps_sx, 1.0 / C)
    mu2 = sb_pool.tile([1, n], FP32, tag=f"lnmu2{tag}")
    nc.vector.tensor_mul(mu2, mu, mu)
    var = sb_pool.tile([1, n], FP32, tag=f"lnvar{tag}")
    nc.vector.scalar_tensor_tensor(var, ps_sx2, 1.0 / C, mu2,
                                   op0=OP.mult, op1=OP.subtract)
    sd = sb_pool.tile([1, n], FP32, tag=f"lnsd{tag}")
    nc.scalar.activation(sd, var, AF.Sqrt, bias=eps_sb[0:1, :])
    rstd = sb_pool.tile([1, n], FP32, tag=f"lnrstd{tag}")
    nc.vector.reciprocal(rstd, sd)
    mu_b = sb_pool.tile([P, n], FP32, tag=f"lnmub{tag}")
    nc.gpsimd.partition_broadcast(mu_b, mu)
    rstd_b = sb_pool.tile([P, n], FP32, tag=f"lnrstdb{tag}")
    nc.gpsimd.partition_broadcast(rstd_b, rstd)
    for cc in range(CCH):
        tmp = sb_pool.tile([P, n], FP32, tag=f"lntmp{tag}")
        nc.vector.tensor_sub(tmp, xT_sb[:, cc, :], mu_b)
        nc.vector.tensor_mul(tmp, tmp, rstd_b)
        nc.vector.tensor_scalar(out_sb[:, cc, :], tmp,
                                lnw_sb[:, cc:cc + 1], lnb_sb[:, cc:cc + 1],
                                op0=OP.mult, op1=OP.add)


def _bcast_mid(ap2d, reps):
    """(p, f) AP -> (p, reps, f) AP with a step-0 middle dim."""
    pairs = list(ap2d.ap)
    assert len(pairs) == 2
    return bass.AP(tensor=ap2d.tensor, offset=ap2d.offset,
                   ap=[list(pairs[0]), [0, reps], list(pairs[1])])


def _bcast_inner(ap2d, reps):
    """(p, f) AP -> (p, f, reps) AP with a step-0 inner dim."""
    pairs = list(ap2d.ap)
    assert len(pairs) == 2
    return bass.AP(tensor=ap2d.tensor, offset=ap2d.offset,
                   ap=[list(pairs[0]), list(pairs[1]), [0, reps]])


def _build_program(sim_gelu=False):
    nc = bacc.Bacc("TRN2", debug=False, num_devices=NC)

    def din(name, shape, dt):
        return nc.dram_tensor(name, shape, dt, kind="ExternalInput").ap()

    xT = din("xT", [C, T], FP32)           # full x, transposed
    xTm = din("xTm", [C, R], FP32)         # this core's columns of xT
    xrows = din("xrows", [R, C], FP32)     # this core's rows of x
    msk = din("msk", [T, E * R], BF16)     # one-hot(bias_matrix.T)*causal, cols (e,i)
    wqT = din("wqT", [C, C], BF16)
    wkT = din("wkT", [C, C], BF16)
    wvT = din("wvT", [C, C], BF16)
    qb = din("qb", [C, 1], FP32)
    kb = din("kb", [C, 1], FP32)
    vbr = din("vbr", [1, C], BF16)
    eeT = din("eeT", [C, E], BF16)
    wekT = din("wekT", [C, C], BF16)
    wevT = din("wevT", [C, C], BF16)
    ekb = din("ekb", [C, 1], FP32)
    evb = din("evb", [C, 1], FP32)
    abr = din("abr", [1, H * E], FP32)   # attn_bias_emb.T flattened
    wpT = din("wpT", [C, C], BF16)
    pbr = din("pbr", [1, C], BF16)
    ln1w = din("ln1w", [C, 1], FP32)
    ln1b = din("ln1b", [C, 1], FP32)
    ln2w = din("ln2w", [C, 1], FP32)
    ln2b = din("ln2b", [C, 1], FP32)
    cfcT = din("cfcT", [C, F], BF16)
    fcbr = din("fcbr", [1, F], BF16)
    cprojT = din("cprojT", [F, C], BF16)
    cpbr = din("cpbr", [1, C], BF16)
    out = nc.dram_tensor("out", [R, C], FP32, kind="ExternalOutput").ap()

    with tile.TileContext(nc) as tc:
        with (
            tc.tile_pool(name="w", bufs=1) as wp,          # weights, loaded once
            tc.tile_pool(name="sb", bufs=2) as sb,         # working sbuf tiles
            tc.tile_pool(name="acts", bufs=1) as acts,     # persistent activations
            tc.tile_pool(name="pP", bufs=4) as pP,         # attention P tiles
            tc.tile_pool(name="psS", bufs=4, space="PSUM") as psS,
            tc.tile_pool(name="psY", bufs=2, space="PSUM") as psY,
            tc.tile_pool(name="psM", bufs=2, space="PSUM") as psM,
        ):
            # ---- constants ----
            ones_f32 = wp.tile([P, 1], FP32)
            nc.vector.memset(ones_f32, 1.0)
            ones_bf = wp.tile([1, P], BF16)
            nc.vector.memset(ones_bf, 1.0)
            ident = wp.tile([P, P], FP32)
            make_identity(nc, ident[:, :])
            eps_sb = wp.tile([P, 1], FP32)
            nc.vector.memset(eps_sb, 1e-5)

            # ---- weight loads ----
            def loadT(ap, name):  # (C, n) -> (128, CCH, n)
                return wp.tile_from(ap.rearrange("(cc p) n -> p cc n", p=P),
                                    name=name)

            wq_sb = loadT(wqT, "wq_sb")
            wk_sb = loadT(wkT, "wk_sb")
            wv_sb = loadT(wvT, "wv_sb")
            wek_sb = loadT(wekT, "wek_sb")
            wev_sb = loadT(wevT, "wev_sb")
            wp_sb = wp.tile_from(wpT.rearrange("(h d) n -> d h n", d=D),
                                 name="wp_sb")
            cfc_sb = loadT(cfcT, "cfc_sb")
            cproj_sb = wp.tile_from(
                cprojT.rearrange("(rc p) n -> p rc n", p=P), name="cproj_sb")
            ee_sb = loadT(eeT, "ee_sb")

            def loadv(ap, name):  # (C,1) f32 -> (128, CCH)
                return wp.tile_from(ap.rearrange("(cc p) one -> p (cc one)", p=P),
                                    name=name)

            qb_sb = loadv(qb, "qb_sb")
            kb_sb = loadv(kb, "kb_sb")
            ekb_sb = loadv(ekb, "ekb_sb")
            evb_sb = loadv(evb, "evb_sb")
            ln1w_sb = loadv(ln1w, "ln1w_sb")
            ln1b_sb = loadv(ln1b, "ln1b_sb")
            ln2w_sb = loadv(ln2w, "ln2w_sb")
            ln2b_sb = loadv(ln2b, "ln2b_sb")
            vbr_sb = wp.tile_from(vbr, name="vbr_sb")
            pbr_sb = wp.tile_from(pbr, name="pbr_sb")
            fcbr_sb = wp.tile_from(fcbr, name="fcbr_sb")
            cpbr_sb = wp.tile_from(cpbr, name="cpbr_sb")
            abr_sb = wp.tile_from(abr, name="abr_sb")
            msk_sb = wp.tile_from(msk.rearrange("(jb p) f -> p jb f", p=P),
                                  name="msk_sb")
            xT_sb = wp.tile_from(xT.rearrange("(cc p) n -> p cc n", p=P),
                                 name="xT_sb")
            xTm_sb = wp.tile_from(xTm.rearrange("(cc p) n -> p cc n", p=P),
                                  name="xTm_sb")
            xrows_sb = wp.tile_from(xrows, name="xrows_sb")

            # ---- LN1 (transposed layout), full and own-rows ----
            hT = acts.tile([P, CCH, T], BF16)      # LN1(x)^T, for K and V
            hTm = acts.tile([P, CCH, R], BF16)     # LN1(x)^T own cols, for Q
            _ln_transposed(nc, (psM, sb), xT_sb, T, ln1w_sb, ln1b_sb, hT,
                           ones_f32, eps_sb, "f")
            _ln_transposed(nc, (psM, sb), xTm_sb, R, ln1w_sb, ln1b_sb, hTm,
                           ones_f32, eps_sb, "m")

            # ---- Q^T (C,R), K^T (C,T) ----
            qT = acts.tile([P, CCH, R], BF16)
            kT = acts.tile([P, CCH, T], BF16)
            for rc in range(CCH):
                ps_q = psM.tile([P, R], FP32, tag="m")
                for cc in range(CCH):
                    nc.tensor.matmul(ps_q, wq_sb[:, cc, rc * P:(rc + 1) * P],
                                     hTm[:, cc, :],
                                     start=(cc == 0), stop=(cc == CCH - 1))
                nc.vector.tensor_scalar(qT[:, rc, :], ps_q,
                                        qb_sb[:, rc:rc + 1], None, op0=OP.add)
                ps_k = psM.tile([P, T], FP32, tag="m")
                for cc in range(CCH):
                    nc.tensor.matmul(ps_k, wk_sb[:, cc, rc * P:(rc + 1) * P],
                                     hT[:, cc, :],
                                     start=(cc == 0), stop=(cc == CCH - 1))
                nc.vector.tensor_scalar(kT[:, rc, :], ps_k,
                                        kb_sb[:, rc:rc + 1], None, op0=OP.add)

            # ---- V (j,d) layout, augmented with ones column: (128, jb, h, 65) ----
            v_aug = acts.tile([P, NJB, H, D + 1], BF16)
            for jb in range(NJB):
                ps_v = psM.tile([P, C], FP32, tag="m")
                for cc in range(CCH):
                    nc.tensor.matmul(ps_v, hT[:, cc, jb * P:(jb + 1) * P],
                                     wv_sb[:, cc, :],
                                     start=(cc == 0), stop=False)
                nc.tensor.matmul(ps_v, ones_bf, vbr_sb, start=False, stop=True)
                nc.vector.tensor_copy(
                    v_aug[:, jb, :, 0:D],
                    ps_v.rearrange("p (h d) -> p h d", h=H))
                nc.vector.memset(v_aug[:, jb, :, D:D + 1], 1.0)

            # ---- edge tables tab_k^T, tab_v^T (C,E); scalv (65,E) per head ----
            tabk = acts.tile([P, CCH, E], FP32)
            for rc in range(CCH):
                ps_t = psM.tile([P, E], FP32, tag="m")
                for cc in range(CCH):
                    nc.tensor.matmul(ps_t, wek_sb[:, cc, rc * P:(rc + 1) * P],
                                     ee_sb[:, cc, :],
                                     start=(cc == 0), stop=(cc == CCH - 1))
                nc.vector.tensor_scalar(tabk[:, rc, :], ps_t,
                                        ekb_sb[:, rc:rc + 1], None, op0=OP.add)
            # tab_v in head-aligned (64, H, E) layout (base partition 0 for all h)
            evb2_sb = wp.tile_from(evb.rearrange("(h d) one -> d (h one)", d=D),
                                   name="evb2_sb")
            tabv = acts.tile([D, H, E], FP32)
            for h in range(H):
                ps_t = psM.tile([D, E], FP32, tag="m")
                for cc in range(CCH):
                    nc.tensor.matmul(ps_t, wev_sb[:, cc, h * D:(h + 1) * D],
                                     ee_sb[:, cc, :],
                                     start=(cc == 0), stop=(cc == CCH - 1))
                nc.vector.tensor_scalar(tabv[:, h, :], ps_t,
                                        evb2_sb[:, h:h + 1], None, op0=OP.add)

            expab = sb.tile([1, H * E], FP32, tag="expab")
            nc.scalar.activation(expab, abr_sb, AF.Exp)
            scalv = acts.tile([D + 1, H, E], FP32)
            for h in range(H):
                nc.gpsimd.partition_broadcast(scalv[:, h, :],
                                              expab[0:1, h * E:(h + 1) * E])
                nc.vector.tensor_mul(
                    scalv[0:D, h, :], scalv[0:D, h, :], tabv[:, h, :])

            # ---- attention ----
            ynT = acts.tile([D, H, R], BF16)      # normalized head outputs
            for hp in range(H // 2):              # head pairs share a 128-part tile
                q_all = sb.tile([P, E * R], BF16, tag="q_all")
                nc.vector.tensor_tensor(
                    q_all.rearrange("p (e r) -> p e r", e=E),
                    _bcast_mid(qT[:, hp, :], E),
                    _bcast_inner(tabk[:, hp, :], R),
                    op=OP.mult)
                for hh in range(2):
                    h = 2 * hp + hh
                    po = hh * D
                    ps_y0 = psY.tile([D + 1, 384], FP32, tag="y")
                    ps_y1 = psY.tile([D + 1, 384], FP32, tag="y")
                    for jb in range(NJB):
                        ps_s0 = psS.tile([P, 384], FP32, tag="s")
                        ps_s1 = psS.tile([P, 384], FP32, tag="s")
                        kT_sl = kT[po:po + D, hp, jb * P:(jb + 1) * P]
                        nc.tensor.matmul(ps_s0, kT_sl, q_all[po:po + D, 0:384],
                                         start=True, stop=True)
                        nc.tensor.matmul(ps_s1, kT_sl, q_all[po:po + D, 384:768],
                                         start=True, stop=True)
                        p_t = pP.tile([P, E * R], BF16, tag="p_t")
                        nc.scalar.activation(p_t[:, 0:384], ps_s0, AF.Exp,
                                             scale=1.0 / math.sqrt(D))
                        nc.scalar.activation(p_t[:, 384:768], ps_s1, AF.Exp,
                                             scale=1.0 / math.sqrt(D))
                        nc.vector.tensor_mul(p_t, p_t, msk_sb[:, jb, :])
                        v_sl = v_aug[:, jb, h, :]
                        nc.tensor.matmul(ps_y0, v_sl, p_t[:, 0:384],
                                         start=(jb == 0), stop=(jb == NJB - 1))
                        nc.tensor.matmul(ps_y1, v_sl, p_t[:, 384:768],
                                         start=(jb == 0), stop=(jb == NJB - 1))
                    # combine over e with per-(e,h) scales; row D is Z
                    acc = sb.tile([D + 1, R], FP32, tag="acc")
                    for e in range(E):
                        ps_y = ps_y0 if e < 8 else ps_y1
                        sl = ps_y[:, (e % 8) * R:(e % 8) * R + R]
                        if e == 0:
                            nc.vector.tensor_scalar(acc, sl,
                                                    scalv[:, h, 0:1], None,
                                                    op0=OP.mult)
                        else:
                            nc.vector.scalar_tensor_tensor(
                                acc, sl, scalv[:, h, e:e + 1], acc,
                                op0=OP.mult, op1=OP.add)
                    rz = sb.tile([1, R], FP32, tag="rz")
                    nc.vector.reciprocal(rz, acc[D:D + 1, :])
                    rz_b = sb.tile([D, R], FP32, tag="rz_b")
                    nc.gpsimd.partition_broadcast(rz_b, rz)
                    nc.vector.tensor_mul(ynT[:, h, :], acc[0:D, :], rz_b)

            # ---- output projection + residual ----
            ps_p = psM.tile([R, C], FP32, tag="m")
            for h in range(H):
                nc.tensor.matmul(ps_p, ynT[:, h, :], wp_sb[:, h, :],
                                 start=(h == 0), stop=False)
            nc.tensor.matmul(ps_p, ones_bf[0:1, 0:R], pbr_sb,
                             start=False, stop=True)
            x2 = acts.tile([R, C], FP32)
            nc.vector.tensor_add(x2, xrows_sb, ps_p)

            # ---- LN2 (row layout) + transpose ----
            st = sb.tile([R, nc.vector.BN_STATS_DIM], FP32, tag="st")
            nc.vector.bn_stats(st, x2)
            mv = sb.tile([R, nc.vector.BN_AGGR_DIM], FP32, tag="mv")
            nc.vector.bn_aggr(mv, st)
            sd2 = sb.tile([R, 1], FP32, tag="sd2")
            nc.scalar.activation(sd2, mv[:, 1:2], AF.Sqrt, bias=eps_sb[0:R, :])
            rstd2 = sb.tile([R, 1], FP32, tag="rstd2")
            nc.vector.reciprocal(rstd2, sd2)
            t2 = sb.tile([R, C], FP32, tag="t2")
            nc.vector.tensor_scalar(t2, x2, mv[:, 0:1], rstd2,
                                    op0=OP.subtract, op1=OP.mult)
            ln2T = acts.tile([P, CCH, R], BF16)
            for cc in range(CCH):
                ps_tr = psM.tile([P, R], FP32, tag="m")
                nc.tensor.transpose(ps_tr, t2[:, cc * P:(cc + 1) * P],
                                    ident[0:R, 0:R])
                nc.vector.tensor_scalar(ln2T[:, cc, :], ps_tr,
                                        ln2w_sb[:, cc:cc + 1],
                                        ln2b_sb[:, cc:cc + 1],
                                        op0=OP.mult, op1=OP.add)

            # ---- MLP ----
            h2T = acts.tile([P, NRC, R], BF16)
            for rc in range(NRC):
                ps_h2 = psM.tile([P, R], FP32, tag="m")
                for cc in range(CCH):
                    nc.tensor.matmul(ps_h2, cfc_sb[:, cc, rc * P:(rc + 1) * P],
                                     ln2T[:, cc, :],
                                     start=(cc == 0), stop=False)
                nc.tensor.matmul(ps_h2, fcbr_sb[0:1, rc * P:(rc + 1) * P],
                                 ones_bf[0:1, 0:R], start=False, stop=True)
                if not sim_gelu:
                    nc.scalar.activation(h2T[:, rc, :], ps_h2, AF.Gelu)
                else:
                    # CoreSim lacks Gelu: tanh-approx (hw uses the exact LUT)
                    h2f = sb.tile([P, R], FP32, tag="h2f")
                    nc.vector.tensor_copy(h2f, ps_h2)
                    sq = sb.tile([P, R], FP32, tag="sq")
                    nc.scalar.square(sq, ps_h2)
                    u = sb.tile([P, R], FP32, tag="u")
                    nc.vector.tensor_scalar(u, sq, 0.035677408136300125,
                                            0.7978845608028654,
                                            op0=OP.mult, op1=OP.add)
                    nc.vector.tensor_mul(u, u, h2f)
                    w = sb.tile([P, R], FP32, tag="wg")
                    nc.scalar.activation(w, u, AF.Tanh)
                    nc.vector.scalar_tensor_tensor(w, w, 1.0, h2f,
                                                   op0=OP.add, op1=OP.mult)
                    nc.vector.tensor_scalar_mul(h2T[:, rc, :], w, 0.5)
            ps_o = psM.tile([R, C], FP32, tag="m")
            for rc in range(NRC):
                nc.tensor.matmul(ps_o, h2T[:, rc, :], cproj_sb[:, rc, :],
                                 start=(rc == 0), stop=False)
            nc.tensor.matmul(ps_o, ones_bf[0:1, 0:R], cpbr_sb,
                             start=False, stop=True)
            out_sb = sb.tile([R, C], FP32, tag="out_sb")
            nc.vector.tensor_add(out_sb, x2, ps_o)
            nc.sync.dma_start(out=out, in_=out_sb)

    nc.compile()
    return nc


def get_program(sim_gelu=False):
    key = ("sim" if sim_gelu else "hw")
    if key not in _prog_cache:
        _prog_cache[key] = _build_program(sim_gelu=sim_gelu)
    return _prog_cache[key]


def make_in_maps(inputs):
    """Host-side sharding/preprocessing. Returns list of 8 input dicts."""
    x = np.asarray(inputs["x"], np.float32)[0]                # (T, C)
    bm = np.asarray(inputs["bias_matrix"], np.int64)[0]       # (T, T)
    w_attn_w = np.asarray(inputs["w_attn_w"], np.float32)
    w_attn_b = np.asarray(inputs["w_attn_b"], np.float32)
    bf = lambda a: np.ascontiguousarray(a, dtype=np.float32).astype(BF16_NP)
    f32 = lambda a: np.ascontiguousarray(a, dtype=np.float32)

    xT = f32(x.T)
    shared = {
        "xT": xT,
        "wqT": bf(w_attn_w[0:C].T),
        "wkT": bf(w_attn_w[C:2 * C].T),
        "wvT": bf(w_attn_w[2 * C:3 * C].T),
        "qb": f32(w_attn_b[0:C].reshape(C, 1)),
        "kb": f32(w_attn_b[C:2 * C].reshape(C, 1)),
        "vbr": bf(w_attn_b[2 * C:3 * C].reshape(1, C)),
        "eeT": bf(np.asarray(inputs["edge_emb"], np.float32).T),
        "wekT": bf(np.asarray(inputs["w_edge_k_w"], np.float32).T),
        "wevT": bf(np.asarray(inputs["w_edge_v_w"], np.float32).T),
        "ekb": f32(np.asarray(inputs["w_edge_k_b"], np.float32).reshape(C, 1)),
        "evb": f32(np.asarray(inputs["w_edge_v_b"], np.float32).reshape(C, 1)),
        "abr": f32(np.asarray(inputs["attn_bias_emb"], np.float32).T.reshape(1, H * E)),
        "wpT": bf(np.asarray(inputs["w_proj_w"], np.float32).T),
        "pbr": bf(np.asarray(inputs["w_proj_b"], np.float32).reshape(1, C)),
        "ln1w": f32(np.asarray(inputs["ln1_w"], np.float32).reshape(C, 1)),
        "ln1b": f32(np.asarray(inputs["ln1_b"], np.float32).reshape(C, 1)),
        "ln2w": f32(np.asarray(inputs["ln2_w"], np.float32).reshape(C, 1)),
        "ln2b": f32(np.asarray(inputs["ln2_b"], np.float32).reshape(C, 1)),
        "cfcT": bf(np.asarray(inputs["c_fc_w"], np.float32).T),
        "fcbr": bf(np.asarray(inputs["c_fc_b"], np.float32).reshape(1, F)),
        "cprojT": bf(np.asarray(inputs["c_proj_w"], np.float32).T),
        "cpbr": bf(np.asarray(inputs["c_proj_b"], np.float32).reshape(1, C)),
    }

    jj = np.arange(T)[:, None]          # j index (column of mask rows)
    in_maps = []
    for c in range(NC):
        rows = np.arange(c, T, NC)      # this core's i rows (48)
        # mask[j, e*R+ii] = (bm[i,j] == e) & (j <= i), i = rows[ii]
        bm_c = bm[rows].T               # (T=j, R=i)
        causal = (jj <= rows[None, :])  # (T, R)
        m = np.zeros((T, E * R), BF16_NP)
        for e in range(E):
            m[:, e * R:(e + 1) * R] = ((bm_c == e) & causal).astype(BF16_NP)
        d = dict(shared)
        d["xTm"] = f32(xT[:, rows])
        d["xrows"] = f32(x[rows])
        d["msk"] = m
        in_maps.append(d)
    return in_maps


def assemble(results):
    out = np.zeros((T, C), np.float32)
    for c in range(NC):
        out[np.arange(c, T, NC)] = results[c]["out"]
    return out.reshape(B, T, C)


def kernel(**inputs):
    nc = get_program()
    in_maps = make_in_maps(inputs)
    res = run_bass_kernel_spmd(nc, in_maps, core_ids=list(range(NC)))
    return assemble(res.results)


if __name__ == "__main__":
    import reference
    ins = reference.setup_inputs()
    ins = {k: np.asarray(v) for k, v in ins.items()}
    exp = np.asarray(reference.reference(**ins))
    got = kernel(**ins)
    err = np.abs(got - exp).max() / np.abs(exp).max()
    print("Relative error:", err)
